# revision 1
# baseline (speedup 1.0000x reference)
"""Trainium2 Bass kernel for nn_CostMapLayer (segment scatter-min + count).

Contract: kernel(**inputs) takes FULL inputs (coords [8,1000000,2] f32,
costs [8,1000000] f32, default_cost [1] f32, H=512, W=512) and returns
(cost [8,512,512] f32, cost_mask [8,512,512] int32), matching reference.

Sharding: data-parallel over batch B=8 -> one batch per NeuronCore.

Device algorithm (per core): maintain a DRAM table [cell, (min,count)] and
stream 128-point chunks: indirect-DMA gather rows for the chunk's cells,
dedup within the chunk via PE transpose + is_equal selection matrix
(min/count across same-cell chunk points), merge, indirect-DMA scatter
back. Iterations serialize through the table (read-modify-write).
"""

import os
import sys
from contextlib import ExitStack

import numpy as np

P = 128
H = 512
W = 512
NCELL = H * W  # 262144
NPAD = 7816  # points per partition after padding (128*7816 = 1000448)
BIG = 3.0e38

_CACHE = {}


def _build(npts_per_part):
    import concourse.bass as bass
    import concourse.tile as tile
    from concourse import bacc, mybir
    from concourse.masks import make_identity

    NP = npts_per_part
    f32 = mybir.dt.float32
    i32 = mybir.dt.int32

    nc = bacc.Bacc("TRN2", target_bir_lowering=False, debug=False, num_devices=8)

    x_d = nc.dram_tensor("x", [P, NP], f32, kind="ExternalInput")
    y_d = nc.dram_tensor("y", [P, NP], f32, kind="ExternalInput")
    v_d = nc.dram_tensor("v", [P, NP], f32, kind="ExternalInput")
    dflt_d = nc.dram_tensor("dflt", [1, 1], f32, kind="ExternalInput")
    outc_d = nc.dram_tensor("out_cost", [P, NCELL // P], f32, kind="ExternalOutput")
    outm_d = nc.dram_tensor("out_mask", [P, NCELL // P], i32, kind="ExternalOutput")
    KTAB = 8
    tables = [
        nc.dram_tensor(f"table{k}", [NCELL + 2, 2], f32, kind="Internal")
        for k in range(KTAB)
    ]

    COLS = NCELL // P  # 2048

    with ExitStack() as ctx:
        tc = ctx.enter_context(tile.TileContext(nc))
        big = ctx.enter_context(tc.tile_pool(name="big", bufs=1))
        sml = ctx.enter_context(tc.tile_pool(name="sml", bufs=8))
        one = ctx.enter_context(tc.tile_pool(name="one", bufs=1))
        psm = ctx.enter_context(tc.tile_pool(name="psm", bufs=3, space="PSUM"))
        psm1 = ctx.enter_context(tc.tile_pool(name="psm1", bufs=1, space="PSUM"))

        # ---- init table: value plane = BIG, count plane = 0 (interleaved) ----
        ROWS_PER_PART = NCELL // P  # 2048 table rows per partition-chunk
        init_iv = big.tile([P, 2 * ROWS_PER_PART], f32, tag="T")
        init_3d = init_iv[:].rearrange("p (a b) -> p a b", b=2)
        nc.vector.memset(init_3d[:, :, 0:1], BIG)
        nc.vector.memset(init_3d[:, :, 1:2], 0.0)
        for kt in range(KTAB):
            t3 = tables[kt].ap()[0:NCELL, :].rearrange(
                "(a b) c -> a b c", b=ROWS_PER_PART
            )
            nc.sync.dma_start(t3, init_iv[:])
            nc.sync.dma_start(
                tables[kt].ap()[NCELL : NCELL + 2, 0:2], init_iv[:2, 0:2]
            )

        # ---- load coords, compute cell ids ----
        X = big.tile([P, NP], f32, tag="X")
        Y = big.tile([P, NP], f32, tag="Y")
        T = big.tile([P, NP], f32, tag="T")
        U = big.tile([P, NP], f32, tag="U")
        CI = big.tile([P, NP], i32, tag="CI")
        nc.sync.dma_start(X[:], x_d.ap())
        nc.sync.dma_start(Y[:], y_d.ap())

        def floor_into(src):
            # src <- floor(src + 0.5), using T/CI as scratch
            nc.vector.tensor_scalar_add(src[:], src[:], 0.5)
            nc.vector.tensor_copy(CI[:], src[:])  # f32 -> i32 cast (unknown rounding)
            nc.vector.tensor_copy(T[:], CI[:])  # back to f32
            nc.vector.tensor_tensor(
                out=U[:], in0=T[:], in1=src[:], op=mybir.AluOpType.is_gt
            )  # rounded up?
            nc.vector.tensor_tensor(
                out=src[:], in0=T[:], in1=U[:], op=mybir.AluOpType.subtract
            )

        floor_into(X)  # X = ix (float, exact integer)
        floor_into(Y)  # Y = iy

        # valid mask in T
        nc.vector.tensor_scalar(
            out=T[:], in0=X[:], scalar1=0.0, scalar2=None, op0=mybir.AluOpType.is_ge
        )
        nc.vector.tensor_scalar(
            out=U[:], in0=X[:], scalar1=float(W), scalar2=None,
            op0=mybir.AluOpType.is_lt,
        )
        nc.vector.tensor_tensor(out=T[:], in0=T[:], in1=U[:], op=mybir.AluOpType.mult)
        nc.vector.tensor_scalar(
            out=U[:], in0=Y[:], scalar1=0.0, scalar2=None, op0=mybir.AluOpType.is_ge
        )
        nc.vector.tensor_tensor(out=T[:], in0=T[:], in1=U[:], op=mybir.AluOpType.mult)
        nc.vector.tensor_scalar(
            out=U[:], in0=Y[:], scalar1=float(H), scalar2=None,
            op0=mybir.AluOpType.is_lt,
        )
        nc.vector.tensor_tensor(out=T[:], in0=T[:], in1=U[:], op=mybir.AluOpType.mult)

        # clip ix/iy to [0, 511] so cell stays in-range even for invalid pts
        nc.vector.tensor_scalar(
            out=X[:], in0=X[:], scalar1=0.0, scalar2=float(W - 1),
            op0=mybir.AluOpType.max, op1=mybir.AluOpType.min,
        )
        nc.vector.tensor_scalar(
            out=Y[:], in0=Y[:], scalar1=0.0, scalar2=float(H - 1),
            op0=mybir.AluOpType.max, op1=mybir.AluOpType.min,
        )

        # cell = iy*512 + ix  (exact in f32), invalid -> NCELL
        nc.vector.tensor_scalar(
            out=U[:], in0=Y[:], scalar1=float(W), scalar2=None,
            op0=mybir.AluOpType.mult,
        )
        nc.vector.tensor_tensor(out=U[:], in0=U[:], in1=X[:], op=mybir.AluOpType.add)
        # U = valid ? cell : NCELL  ==  T*(U - NCELL) + NCELL
        nc.vector.tensor_scalar_add(U[:], U[:], -float(NCELL))
        nc.vector.tensor_tensor(out=U[:], in0=U[:], in1=T[:], op=mybir.AluOpType.mult)
        nc.vector.tensor_scalar_add(U[:], U[:], float(NCELL))
        nc.vector.tensor_copy(CI[:], U[:])  # exact integer cast

        # values: load into X's slot (X no longer needed)
        V = X
        nc.sync.dma_start(V[:], v_d.ap())

        # constants for the loop
        ident = one.tile([P, P], f32)
        make_identity(nc, ident[:])

        # ---- main loop over 128-point chunks ----
        def body(i, kt=0):
            ci_col = sml.tile([P, 1], i32, tag="ci")
            cf_col = sml.tile([P, 1], f32, tag="cf")
            v_col = sml.tile([P, 1], f32, tag="vc")
            nc.vector.tensor_copy(ci_col[:], CI[:, bass.ts(i, 1)])
            nc.vector.tensor_copy(cf_col[:], U[:, bass.ts(i, 1)])
            nc.vector.tensor_copy(v_col[:], V[:, bass.ts(i, 1)])

            rows = sml.tile([P, 2], f32, tag="rows")
            nc.gpsimd.indirect_dma_start(
                out=rows[:],
                out_offset=None,
                in_=tables[kt].ap(),
                in_offset=bass.IndirectOffsetOnAxis(ap=ci_col[:, :1], axis=0),
            )

            cT = psm.tile([P, P], f32, space="PSUM", tag="cT")
            nc.tensor.transpose(
                out=cT[:], in_=cf_col[:].to_broadcast([P, P]), identity=ident[:]
            )
            vT = psm.tile([P, P], f32, space="PSUM", tag="vT")
            nc.tensor.transpose(
                out=vT[:], in_=v_col[:].to_broadcast([P, P]), identity=ident[:]
            )
            sel = sml.tile([P, P], f32, tag="sel")
            nc.vector.tensor_tensor(
                out=sel[:],
                in0=cf_col[:].to_broadcast([P, P]),
                in1=cT[:],
                op=mybir.AluOpType.is_equal,
            )
            msk = sml.tile([P, P], f32, tag="msk")
            t2 = sml.tile([P, P], f32, tag="t2")
            nc.vector.tensor_tensor(
                out=msk[:], in0=sel[:], in1=vT[:], op=mybir.AluOpType.mult
            )
            nc.vector.tensor_scalar(
                out=t2[:], in0=sel[:], scalar1=-BIG, scalar2=BIG,
                op0=mybir.AluOpType.mult, op1=mybir.AluOpType.add,
            )
            nc.vector.tensor_tensor(
                out=msk[:], in0=msk[:], in1=t2[:], op=mybir.AluOpType.add
            )
            minv = sml.tile([P, 1], f32, tag="minv")
            nc.vector.tensor_reduce(
                minv[:], msk[:], axis=mybir.AxisListType.X, op=mybir.AluOpType.min
            )
            cnt = sml.tile([P, 1], f32, tag="cnt")
            nc.vector.tensor_reduce(
                cnt[:], sel[:], axis=mybir.AxisListType.X, op=mybir.AluOpType.add
            )
            rows2 = sml.tile([P, 2], f32, tag="rows2")
            nc.vector.tensor_tensor(
                out=rows2[:, 0:1], in0=rows[:, 0:1], in1=minv[:],
                op=mybir.AluOpType.min,
            )
            nc.vector.tensor_tensor(
                out=rows2[:, 1:2], in0=rows[:, 1:2], in1=cnt[:],
                op=mybir.AluOpType.add,
            )
            nc.gpsimd.indirect_dma_start(
                out=tables[kt].ap(),
                out_offset=bass.IndirectOffsetOnAxis(ap=ci_col[:, :1], axis=0),
                in_=rows2[:],
                in_offset=None,
            )

        def unrollable_body(iv0, unroll):
            for u in range(unroll):
                body(iv0 + u, kt=u % KTAB)

        tc.For_i_unrolled_general(
            start=0, end=NP, step=1, unrollable_body=unrollable_body, max_unroll=KTAB
        )

        # ---- epilogue: build outputs from table ----
        back_iv = big.tile([P, 2 * COLS], f32, tag="T")
        acc_iv = big.tile([P, 2 * COLS], f32, tag="U")
        for kt in range(KTAB):
            t3 = tables[kt].ap()[0:NCELL, :].rearrange(
                "(a b) c -> a b c", b=ROWS_PER_PART
            )
            if kt == 0:
                nc.sync.dma_start(acc_iv[:], t3)
            else:
                nc.sync.dma_start(back_iv[:], t3)
                a3 = acc_iv[:].rearrange("p (a b) -> p a b", b=2)
                b3 = back_iv[:].rearrange("p (a b) -> p a b", b=2)
                nc.vector.tensor_tensor(
                    out=a3[:, :, 0:1], in0=a3[:, :, 0:1], in1=b3[:, :, 0:1],
                    op=mybir.AluOpType.min,
                )
                nc.vector.tensor_tensor(
                    out=a3[:, :, 1:2], in0=a3[:, :, 1:2], in1=b3[:, :, 1:2],
                    op=mybir.AluOpType.add,
                )
        back_3d = acc_iv[:].rearrange("p (a b) -> p a b", b=2)
        valp = big.tile([P, COLS], f32, tag="X")
        cntp = big.tile([P, COLS], f32, tag="Y")
        nc.vector.tensor_copy(valp[:].rearrange("p (a b) -> p a b", b=1), back_3d[:, :, 0:1])
        nc.vector.tensor_copy(cntp[:].rearrange("p (a b) -> p a b", b=1), back_3d[:, :, 1:2])

        # default broadcast to [P,1] via K=1 matmul with ones
        dflt_sb = one.tile([1, 1], f32)
        nc.sync.dma_start(dflt_sb[:], dflt_d.ap())
        ones_row = one.tile([1, P], f32)
        nc.vector.memset(ones_row[:], 1.0)
        dflt_ps = psm1.tile([P, 1], f32, space="PSUM", tag="dflt")
        nc.tensor.matmul(
            dflt_ps[:], lhsT=ones_row[:], rhs=dflt_sb[:], start=True, stop=True
        )
        dflt_col = one.tile([P, 1], f32)
        nc.vector.tensor_copy(dflt_col[:], dflt_ps[:])

        occ = big.tile([P, COLS], f32, tag="U")
        nc.vector.tensor_scalar(
            out=occ[:], in0=cntp[:], scalar1=0.5, scalar2=None,
            op0=mybir.AluOpType.is_gt,
        )
        # out_cost = occ*(valp - dflt) + dflt   (in place on valp)
        nc.vector.tensor_scalar(
            out=valp[:], in0=valp[:], scalar1=dflt_col[:, :1], scalar2=None,
            op0=mybir.AluOpType.subtract,
        )
        nc.vector.tensor_tensor(
            out=valp[:], in0=valp[:], in1=occ[:], op=mybir.AluOpType.mult
        )
        nc.vector.tensor_scalar(
            out=valp[:], in0=valp[:], scalar1=dflt_col[:, :1], scalar2=None,
            op0=mybir.AluOpType.add,
        )
        nc.sync.dma_start(outc_d.ap(), valp[:])

        outm = big.tile([P, COLS], i32, tag="CI")
        nc.vector.tensor_scalar_add(cntp[:], cntp[:], -1.0)
        nc.vector.tensor_copy(outm[:], cntp[:])
        nc.sync.dma_start(outm_d.ap(), outm[:])

    nc.compile()
    return nc


def _get_nc(npts_per_part):
    key = npts_per_part
    if key not in _CACHE:
        _CACHE[key] = _build(npts_per_part)
    return _CACHE[key]


def _prepare_in_maps(coords, costs, default_cost):
    B, N, _ = coords.shape
    npad = P * NPAD
    in_maps = []
    for b in range(B):
        x = np.full(npad, -1.0e6, dtype=np.float32)
        y = np.full(npad, -1.0e6, dtype=np.float32)
        v = np.zeros(npad, dtype=np.float32)
        x[:N] = coords[b, :, 0]
        y[:N] = coords[b, :, 1]
        v[:N] = costs[b]
        in_maps.append(
            {
                "x": x.reshape(P, NPAD),
                "y": y.reshape(P, NPAD),
                "v": v.reshape(P, NPAD),
                "dflt": np.asarray(default_cost, dtype=np.float32).reshape(1, 1),
            }
        )
    return in_maps


def kernel(coords, costs, default_cost, H=512, W=512):
    from concourse.bass_utils import run_bass_kernel_spmd

    B = coords.shape[0]
    nc = _get_nc(NPAD)
    in_maps = _prepare_in_maps(coords, costs, default_cost)

    res = run_bass_kernel_spmd(nc, in_maps, core_ids=list(range(B)))
    cost = np.stack(
        [res.results[b]["out_cost"].reshape(512, 512) for b in range(B)]
    )
    mask = np.stack(
        [res.results[b]["out_mask"].reshape(512, 512).astype(np.int32) for b in range(B)]
    )
    return cost, mask

